# revision 8
# baseline (speedup 1.0000x reference)
"""Sparse-attention Trainium2 kernel (8 NeuronCores, SPMD).

Sharding: core = b*4 + q handles batch b, image rows [28q, 28q+28) (3136 pixels).
Launch 1 pools x (7x7 avg -> 16x16 cells) and ctx (16x16 avg -> 7x7) on disjoint
row slices; host concatenates the tiny pooled partials. Launch 2 runs the full
attention pipeline per core on its 3136 pixels.
"""
import sys
sys.path.insert(0, "/opt/trn_rl_repo")
import numpy as np
import concourse.bacc as bacc
import concourse.mybir as mybir
from concourse.tile import TileContext
from concourse.bass_utils import run_bass_kernel_spmd

F32 = mybir.dt.float32
BF16 = mybir.dt.bfloat16
Copy = mybir.ActivationFunctionType.Copy
Exp = mybir.ActivationFunctionType.Exp
X = mybir.AxisListType.X
ALU = mybir.AluOpType

B, C, CC, H, W = 2, 768, 384, 112, 112
G, HD = 12, 64
S2, K2 = 49, 256
N = 28 * 112          # pixels per core
NW = 448              # window width (7 windows)
NSUB = 112            # stationary subtile width (4 per window)
NEG_INF = -1e30


def _new_nc():
    return bacc.Bacc("TRN2", target_bir_lowering=False, debug=False,
                     enable_asserts=False, num_devices=8)


def _build_pool_kernel():
    nc = _new_nc()
    x_sl = nc.dram_tensor("x_sl", [C, N], F32, kind="ExternalInput")
    ctx_sl = nc.dram_tensor("ctx_sl", [CC, 32 * 112], F32, kind="ExternalInput")
    vp = nc.dram_tensor("vp", [C, 64], F32, kind="ExternalOutput")
    cp = nc.dram_tensor("cp", [CC, 14], F32, kind="ExternalOutput")
    with TileContext(nc) as tc:
        with tc.tile_pool(name="sb", bufs=2) as sb:
            for t in range(6):
                xt = sb.tile([128, N], F32, tag="xt")
                nc.sync.dma_start(xt[:], x_sl[128 * t:128 * (t + 1), :])
                t1 = sb.tile([128, 28, 16], F32, tag="t1")
                nc.vector.tensor_reduce(
                    t1[:], xt[:].rearrange("p (h wb wi) -> p h wb wi", h=28, wi=7),
                    axis=X, op=ALU.add)
                t2 = sb.tile([128, 4, 16], F32, tag="t2")
                nc.vector.tensor_reduce(
                    t2[:], t1[:].rearrange("p (hb hi) w -> p hb w hi", hi=7),
                    axis=X, op=ALU.add)
                nc.sync.dma_start(vp[128 * t:128 * (t + 1), :],
                                  t2[:].rearrange("p a b -> p (a b)"))
            for t in range(3):
                ct = sb.tile([128, 32 * 112], F32, tag="ct")
                nc.sync.dma_start(ct[:], ctx_sl[128 * t:128 * (t + 1), :])
                u1 = sb.tile([128, 32, 7], F32, tag="u1")
                nc.vector.tensor_reduce(
                    u1[:], ct[:].rearrange("p (h wb wi) -> p h wb wi", h=32, wi=16),
                    axis=X, op=ALU.add)
                u2 = sb.tile([128, 2, 7], F32, tag="u2")
                nc.vector.tensor_reduce(
                    u2[:], u1[:].rearrange("p (hb hi) w -> p hb w hi", hi=16),
                    axis=X, op=ALU.add)
                nc.sync.dma_start(cp[128 * t:128 * (t + 1), :],
                                  u2[:].rearrange("p a b -> p (a b)"))
    nc.compile()
    return nc


def _build_main_kernel():
    nc = _new_nc()
    x_sl = nc.dram_tensor("x_sl", [C, N], F32, kind="ExternalInput")
    wqT = nc.dram_tensor("wqT", [C, C], F32, kind="ExternalInput")
    wkT = nc.dram_tensor("wkT", [CC, C], F32, kind="ExternalInput")
    wdT = nc.dram_tensor("wdT", [S2, K2], F32, kind="ExternalInput")
    pwT = nc.dram_tensor("pwT", [C, C], F32, kind="ExternalInput")
    pbm = nc.dram_tensor("pbm", [128, 6], F32, kind="ExternalInput")
    v_t = nc.dram_tensor("v_t", [K2, C], F32, kind="ExternalInput")
    cpool = nc.dram_tensor("cpool", [CC, S2], F32, kind="ExternalInput")
    ident = nc.dram_tensor("ident", [NSUB, NSUB], F32, kind="ExternalInput")
    sel2 = nc.dram_tensor("sel2", [2, 128], F32, kind="ExternalInput")
    out = nc.dram_tensor("out", [C, N], F32, kind="ExternalOutput")

    with TileContext(nc) as tc:
        with tc.tile_pool(name="wts", bufs=1) as wts, \
             tc.tile_pool(name="sb", bufs=2) as sb, \
             tc.tile_pool(name="ps", bufs=7, space="PSUM") as ps:

            def cast_in(dram, shape, tag, scale=None):
                f = sb.tile(shape, F32, tag="stage")
                nc.sync.dma_start(f[:], dram)
                b = wts.tile(shape, BF16, tag=tag)
                if scale is None:
                    nc.vector.tensor_copy(b[:], f[:])
                else:
                    nc.scalar.activation(b[:], f[:], Copy, scale=scale)
                return b

            # persistent weights (bf16)
            wq_bf = [cast_in(wqT[128 * t:128 * (t + 1), :], [128, C], f"wq{t}") for t in range(6)]
            pw_bf = [cast_in(pwT[128 * t:128 * (t + 1), :], [128, C], f"pw{t}") for t in range(6)]
            wd_bf = cast_in(wdT[:], [S2, K2], "wd")
            wd_hi_full = wts.tile([128, K2], BF16, tag="wdhi")
            nc.vector.tensor_copy(wd_hi_full[64:64 + S2, :], wd_bf[:])
            wd_hi = wd_hi_full[64:64 + S2, :]
            id_bf = cast_in(ident[:], [NSUB, NSUB], "id")
            s2_bf = cast_in(sel2[:], [2, 128], "s2")
            pb_sb = wts.tile([128, 6], F32, tag="pb")
            nc.sync.dma_start(pb_sb[:], pbm[:])

            # k = wk @ (ctx_pool/49): [768, 49] bf16, then block-diag pairs k2 [128, 98]
            cp_bf = [cast_in(cpool[128 * t:128 * (t + 1), :], [128, S2], f"cp{t}",
                             scale=1.0 / 49.0) for t in range(3)]
            wk_bf = [cast_in(wkT[128 * t:128 * (t + 1), :], [128, C], f"wk{t}") for t in range(3)]
            k2_bf = []
            for o in range(6):
                pk = ps.tile([128, S2], F32, tag="ps")
                for ci in range(3):
                    nc.tensor.matmul(pk[:], wk_bf[ci][:, 128 * o:128 * (o + 1)],
                                     cp_bf[ci][:], start=(ci == 0), stop=(ci == 2))
                kb = wts.tile([128, S2], BF16, tag=f"k{o}")
                nc.scalar.activation(kb[:], pk[:], Copy)
                k2 = wts.tile([128, 2 * S2], BF16, tag=f"k2_{o}")
                nc.vector.memset(k2[:], 0.0)
                nc.vector.tensor_copy(k2[0:64, 0:S2], kb[0:64, :])
                nc.vector.tensor_copy(k2[64:128, S2:2 * S2], kb[64:128, :])
                k2_bf.append(k2)

            # v_aug [128, 12, 65] per k-half: cols 0:64 v^T/49 per head, col 64 ones
            v_aug = []
            for kh in range(2):
                vb = cast_in(v_t[128 * kh:128 * (kh + 1), :], [128, C], f"v{kh}",
                             scale=1.0 / 49.0)
                va = wts.tile([128, G, 65], BF16, tag=f"va{kh}")
                nc.vector.memset(va[:], 1.0)
                nc.vector.tensor_copy(
                    va[:].rearrange("p g a -> p g a")[:, :, 0:64],
                    vb[:].rearrange("p (g a) -> p g a", a=64))
                v_aug.append(va)

            for w in range(7):
                cw = slice(NW * w, NW * (w + 1))
                # x window + bf16 cast + x+pb precompute
                x_f, x_bf, xpb = [], [], []
                for t in range(6):
                    xf = sb.tile([128, NW], F32, tag=f"x{t}")
                    nc.sync.dma_start(xf[:], x_sl[128 * t:128 * (t + 1), cw])
                    xb = sb.tile([128, NW], BF16, tag=f"xb{t}")
                    nc.vector.tensor_copy(xb[:], xf[:])
                    xp = sb.tile([128, NW], F32, tag=f"xp{t}")
                    nc.vector.tensor_scalar(xp[:], xf[:], pb_sb[:, t:t + 1], None,
                                            op0=ALU.add)
                    x_f.append(xf); x_bf.append(xb); xpb.append(xp)

                # q projection (scaled by hd^-0.5 on eviction)
                q_bf = []
                for o in range(6):
                    pq = ps.tile([128, NW], F32, tag="ps")
                    for ci in range(6):
                        nc.tensor.matmul(pq[:], wq_bf[ci][:, 128 * o:128 * (o + 1)],
                                         x_bf[ci][:], start=(ci == 0), stop=(ci == 5))
                    qb = sb.tile([128, NW], BF16, tag=f"q{o}")
                    nc.scalar.activation(qb[:], pq[:], Copy, scale=float(HD) ** -0.5)
                    q_bf.append(qb)

                oa_bf = []
                for t in range(6):
                    # attention scores for head pair t: [112, 4, 2, 49]
                    pa = ps.tile([NSUB, 4, 2 * S2], F32, tag="ps")
                    for s in range(4):
                        nc.tensor.matmul(pa[:, s, :],
                                         q_bf[t][:, NSUB * s:NSUB * (s + 1)],
                                         k2_bf[t][:])
                    at = sb.tile([NSUB, 4, 2, S2], F32, tag="at")
                    nc.scalar.activation(at[:], pa[:].rearrange("p a (b s) -> p a b s", s=S2), Copy)

                    # top-32 of 49: find 17th-smallest threshold via negated max8
                    ng = sb.tile([NSUB, 4, 2, S2], F32, tag="ng")
                    nc.vector.tensor_scalar_mul(ng[:], at[:], -1.0)
                    for s in range(4):
                        for hh in range(2):
                            m8 = sb.tile([NSUB, 8], F32, tag="m8")
                            sl = ng[:, s, hh, :]
                            nc.vector.max(out=m8[:], in_=sl)
                            nc.vector.match_replace(out=sl, in_to_replace=m8[:],
                                                    in_values=sl, imm_value=NEG_INF)
                            m8b = sb.tile([NSUB, 8], F32, tag="m8")
                            nc.vector.max(out=m8b[:], in_=sl)
                            nc.vector.match_replace(out=sl, in_to_replace=m8b[:],
                                                    in_values=sl, imm_value=NEG_INF)
                    tau = sb.tile([NSUB, 8], F32, tag="tau")
                    nc.vector.tensor_reduce(tau[:], ng[:].rearrange("p a b s -> p (a b) s"),
                                            axis=X, op=ALU.max)
                    nthr = sb.tile([NSUB, 8], F32, tag="nthr")
                    nc.vector.tensor_scalar_mul(nthr[:], tau[:], -1.0)
                    msk = sb.tile([NSUB, 4, 2, S2], F32, tag="msk")
                    nc.vector.tensor_tensor(
                        out=msk[:].rearrange("p a b s -> p (a b) s"),
                        in0=at[:].rearrange("p a b s -> p (a b) s"),
                        in1=nthr[:].to_broadcast((NSUB, 8, S2)), op=ALU.is_gt)
                    # sparse (bf16) into zero-padded [112, 4, 2, 64] for transpose
                    sp = sb.tile([NSUB, 4, 2, 64], BF16, tag="sp")
                    nc.gpsimd.memset(sp[:], 0.0)
                    nc.vector.tensor_tensor(out=sp[:, :, :, 0:S2], in0=at[:], in1=msk[:],
                                            op=ALU.mult)

                    # transpose via identity matmul -> spT [128, 4, 112] bf16
                    pt = ps.tile([128, 4, NSUB], F32, tag="ps")
                    for s in range(4):
                        nc.tensor.matmul(pt[:, s, :],
                                         sp[:, s, :, :].rearrange("p a b -> p (a b)"),
                                         id_bf[:])
                    spT = sb.tile([128, 4, NSUB], BF16, tag="spT")
                    nc.scalar.activation(spT[:], pt[:], Copy)

                    # per-head: omega logits -> exp -> out-mm (with sum row)
                    pouts = []
                    for hh in range(2):
                        base = 64 * hh
                        rhs = spT[base:base + S2, :, :].rearrange("p a b -> p (a b)")
                        po = ps.tile([65, NW], F32, tag="ps")
                        wd_use = wd_bf if hh == 0 else wd_hi
                        for kh in range(2):
                            pm = ps.tile([128, NW], F32, tag="ps")
                            nc.tensor.matmul(pm[:], wd_use[:, 128 * kh:128 * (kh + 1)], rhs)
                            ex = sb.tile([128, NW], BF16, tag="ex")
                            nc.scalar.activation(ex[:], pm[:], Exp)
                            nc.tensor.matmul(po[:], v_aug[kh][:, 2 * t + hh, :], ex[:],
                                             start=(kh == 0), stop=(kh == 1))
                        pouts.append(po)

                    # softmax denominators for the pair -> recip -> replicate
                    sm = sb.tile([2, NW], F32, tag="sm")
                    nc.scalar.activation(sm[0:1, :], pouts[0][64:65, :], Copy)
                    smt = sb.tile([1, NW], F32, tag="smt")
                    nc.scalar.activation(smt[0:1, :], pouts[1][64:65, :], Copy)
                    nc.sync.dma_start(sm[1:2, :], smt[0:1, :])
                    rc = sb.tile([2, NW], F32, tag="rc")
                    nc.vector.reciprocal(rc[:], sm[:])
                    rcb = sb.tile([2, NW], BF16, tag="rcb")
                    nc.vector.tensor_copy(rcb[:], rc[:])
                    pr = ps.tile([128, NW], F32, tag="ps")
                    nc.tensor.matmul(pr[:], s2_bf[:], rcb[:])
                    rep = sb.tile([128, NW], F32, tag="rep")
                    nc.scalar.activation(rep[:], pr[:], Copy)

                    oa = sb.tile([128, NW], BF16, tag=f"oa{t}")
                    for hh in range(2):
                        nc.vector.tensor_tensor(out=oa[64 * hh:64 * (hh + 1), :],
                                                in0=pouts[hh][0:64, :],
                                                in1=rep[64 * hh:64 * (hh + 1), :],
                                                op=ALU.mult)
                    oa_bf.append(oa)

                # output projection + bias + residual
                for o in range(6):
                    py = ps.tile([128, NW], F32, tag="ps")
                    for ci in range(6):
                        nc.tensor.matmul(py[:], pw_bf[ci][:, 128 * o:128 * (o + 1)],
                                         oa_bf[ci][:], start=(ci == 0), stop=(ci == 5))
                    y = sb.tile([128, NW], F32, tag=f"y{o}")
                    nc.vector.tensor_tensor(out=y[:], in0=py[:], in1=xpb[o][:], op=ALU.add)
                    nc.sync.dma_start(out[128 * o:128 * (o + 1), cw], y[:])
    nc.compile()
    return nc


_CACHE = {}


def kernel(x, ctx, wq, wk, wd, proj_w, proj_b):
    x = np.ascontiguousarray(np.asarray(x, dtype=np.float32))
    ctx = np.ascontiguousarray(np.asarray(ctx, dtype=np.float32))

    if "pool" not in _CACHE:
        _CACHE["pool"] = _build_pool_kernel()
    if "main" not in _CACHE:
        _CACHE["main"] = _build_main_kernel()

    # ---- launch 1: pooling partials
    ctx_starts = [0, 32, 64, 80]
    in1 = []
    for core in range(8):
        b, q = core // 4, core % 4
        xs = x[b, :, 28 * q:28 * (q + 1), :].reshape(C, N)
        cs = ctx[b, :, ctx_starts[q]:ctx_starts[q] + 32, :].reshape(CC, 32 * 112)
        in1.append({"x_sl": np.ascontiguousarray(xs), "ctx_sl": np.ascontiguousarray(cs)})
    r1 = run_bass_kernel_spmd(_CACHE["pool"], in1, list(range(8))).results

    v = np.zeros((B, C, 16, 16), np.float32)
    cpool = np.zeros((B, CC, 7, 7), np.float32)
    for core in range(8):
        b, q = core // 4, core % 4
        v[b, :, 4 * q:4 * (q + 1), :] = r1[core]["vp"].reshape(C, 4, 16)
        cpb = r1[core]["cp"].reshape(CC, 2, 7)
        if q < 3:
            cpool[b, :, 2 * q:2 * q + 2, :] = cpb
        else:
            cpool[b, :, 6, :] = cpb[:, 1, :]

    # ---- launch 2: main pipeline
    wqT = np.ascontiguousarray(wq.T.astype(np.float32))
    wkT = np.ascontiguousarray(wk.T.astype(np.float32))
    wdT = np.ascontiguousarray(wd.T.astype(np.float32))
    pwT = np.ascontiguousarray(proj_w.T.astype(np.float32))
    pbm = np.ascontiguousarray(proj_b.astype(np.float32).reshape(6, 128).T)
    ident = np.eye(NSUB, dtype=np.float32)
    sel2 = np.zeros((2, 128), np.float32)
    sel2[0, :64] = 1.0
    sel2[1, 64:] = 1.0

    in2 = []
    for core in range(8):
        b, q = core // 4, core % 4
        xs = np.ascontiguousarray(x[b, :, 28 * q:28 * (q + 1), :].reshape(C, N))
        vt = np.ascontiguousarray(v[b].reshape(C, K2).T)
        cpl = np.ascontiguousarray(cpool[b].reshape(CC, S2))
        in2.append({"x_sl": xs, "wqT": wqT, "wkT": wkT, "wdT": wdT, "pwT": pwT,
                    "pbm": pbm, "v_t": vt, "cpool": cpl, "ident": ident, "sel2": sel2})
    _CACHE["last_in2"] = in2
    r2 = run_bass_kernel_spmd(_CACHE["main"], in2, list(range(8))).results

    y = np.zeros((B, C, H, W), np.float32)
    for core in range(8):
        b, q = core // 4, core % 4
        y[b, :, 28 * q:28 * (q + 1), :] = r2[core]["out"].reshape(C, 28, 112)
    return y
